# revision 1
# baseline (speedup 1.0000x reference)
"""Trainium2 Bass kernel for masked attention softmax (ragged sequences).

Reference computation (per batch b):
    qp[k]   = sum_q query[b,0,q] * w[k,q]
    att[s]  = sum_k qp[k] * keys[b,s,k]
    score   = where(s < seq_len[b], att, NEG_INF)
    out[b]  = softmax(score)            # over s axis

Strategy:
  - Data-parallel over batch across 8 cores (512 batches/core, 4 tiles of 128).
  - Ragged trick: sort batches by seq_len descending (host-side), deal
    round-robin to cores so tile slot j has the same max length on every
    core; bake that extent into the kernel and only load/compute
    keys[:, :s_ext_j, :].  Saves ~half of the DMA+compute.
  - Mask fused into the data: host appends a 129th key element per (b,s)
    holding 0 (valid) or NEG_INF (masked); qp gets a fixed 1.0 appended, so
    the dot product IS the masked score.
  - Per 128-batch tile (batch on partitions):
      * qp via one PE matmul (query tile pre-transposed on host, fused
        [qT | wT] load so the Matmult needs a single semaphore wait)
      * score via one DVE scalar_tensor_tensor per s position (fused
        multiply + accumulate at 1 elem/cycle; this is the bottleneck at
        ~207ns per position, ~103us/core)
      * qp for all tiles is computed up-front (PE/ACT idle) so it never
        gates the STT stream
      * softmax without max-subtraction (|att| <= ~60 so exp is finite;
        softmax is shift-invariant; seq_len==0 rows give 0/0 and are
        overwritten by the host): ACT exp(accum_out=sum) -> DVE
        reciprocal -> ACT copy(scale=1/sum)
  - Keys streamed in ~3.3MB chunks (HWDGE), geometric ramp-up on the first
    tile so the DVE starts within ~6us.
  - Host scatters per-core outputs back via inverse permutation; rows with
    seq_len == 0 are uniform 1/S (reference softmax of all-equal scores).

  Measured on trn2 (8 cores): ~129.8us HW exec (3 consecutive runs
  129.8-129.9us; occasional contention runs up to ~135us), max rel err
  ~8e-6.  Output DMAs ride SWDGE (gpsimd) so the Sync/HWDGE queue
  carries only keys chunks.
  Rejected alternatives (measured): tensor_tensor_reduce (crashes runtime),
  GpSimd tensor_tensor offload (SBUF contention slows concurrent DVE 4x),
  PE batched-matvec via per-batch stationaries (2-pass fp32 matmul +
  LDWEIGHTS overheads ~610ns/batch), bf16 keys (3.3e-2 abs err).
"""

import sys

import numpy as np

sys.path.insert(0, "/opt/trn_rl_repo")

import concourse.bass as bass
import concourse.tile as tile
from concourse import bacc, mybir
from concourse.bass_utils import run_bass_kernel_spmd


def _install_trace_shims():
    """The agent image lacks ``antenv.axon_hooks``, so trace=True silently
    degrades.  Recreate the module and register the ctypes NTFF hook from
    trn_agent_boot; also make artifact upload failure non-fatal."""
    try:
        import types

        import antenv
        from concourse import bass_utils as _bu

        if "antenv.axon_hooks" not in sys.modules:
            mod = types.ModuleType("antenv.axon_hooks")
            mod._hook = None
            mod.set_axon_ntff_profile_hook = lambda h: setattr(mod, "_hook", h)
            mod.get_axon_ntff_profile_hook = lambda: mod._hook
            sys.modules["antenv.axon_hooks"] = mod
            antenv.axon_hooks = mod
            from trn_agent_boot.trn_boot import _ntff_profile_via_ctypes

            mod.set_axon_ntff_profile_hook(
                _ntff_profile_via_ctypes("/opt/axon/libaxon_pjrt.so")
            )

        _orig_upload = _bu.upload_artifacts

        def _safe_upload(tmpdir):
            try:
                return _orig_upload(tmpdir)
            except Exception:
                return "local://" + str(tmpdir)

        _bu.upload_artifacts = _safe_upload
    except Exception:
        pass


_install_trace_shims()

B, S, KD, QD = 4096, 200, 128, 128
NCORES = 8
P = 128
PB = B // NCORES           # batches per core
NTILES = PB // P           # partition tiles per core
NEG_INF = float(-(2**32) + 1)
CH = 50                    # s-positions per keys DMA chunk
KDA = KD + 1               # keys augmented with a mask-penalty element

LAST_RESULTS = None
_nc_cache = {}


def _round8(x):
    return ((int(x) + 7) // 8) * 8


def _build(s_exts):
    f32 = mybir.dt.float32
    # Bacc (not raw Bass): its compile() pass splits multi-semaphore waits
    # into EventSemaphore instructions (TRN2 allows <=1 wait per instruction)
    # and moves matmul waits onto ldweights.
    nc = bacc.Bacc("TRN2", target_bir_lowering=False, debug=False)
    # keys augmented host-side with a 129th element = 0 (s < len) or
    # NEG_INF (masked); qp gets a fixed 1.0 appended, so the fused STT
    # accumulate yields the masked score directly.
    keys_d = nc.dram_tensor("keys", [PB, S, KDA], f32, kind="ExternalInput")
    # qw[j] = [qT_j | wT] fused so each tile's matmul depends on ONE dma
    # (walrus limits sync-wait commands on Matmult/LDWEIGHTS).
    qw_d = nc.dram_tensor("qw", [QD, NTILES, P + KD], f32, kind="ExternalInput")
    out_d = nc.dram_tensor("out", [PB, S], f32, kind="ExternalOutput")

    with tile.TileContext(nc) as tc:
        with (
            tc.tile_pool(name="keys", bufs=4) as keysp,
            tc.tile_pool(name="small", bufs=2) as smallp,
            tc.tile_pool(name="qpp", bufs=NTILES) as qpp,
            tc.tile_pool(name="scr", bufs=16) as scrp,
            tc.tile_pool(name="psum", bufs=4, space=bass.MemorySpace.PSUM) as psump,
        ):
            # qp for ALL tiles up-front via ONE fused qw DMA (a single Sync
            # issue, so the first keys chunk isn't queued behind 4 issues);
            # PE/ACT are otherwise idle, so every tile's qp is ready long
            # before its first STT -- qp never sits on the critical path.
            qw = smallp.tile([QD, NTILES, P + KD], f32, tag="qw")
            nc.sync.dma_start(qw[:], qw_d[:])
            qps = []
            for j in range(NTILES):
                # qp[b,k] = sum_q qT[q,b] * wT[q,k]; qp[:,128] = 1.0 so the
                # augmented key element contributes the mask penalty.
                qp_ps = psump.tile([P, KD], f32, tag="qp_ps")
                nc.tensor.matmul(
                    qp_ps[:], qw[:, j, :P], qw[:, j, P : P + KD],
                    start=True, stop=True,
                )
                qp = qpp.tile([P, KDA], f32, tag=f"qp{j}")
                nc.gpsimd.memset(qp[:, KD : KD + 1], 1.0)
                nc.scalar.copy(qp[:, :KD], qp_ps[:])
                qps.append(qp)

            kt0 = keysp.tile([P, CH, KDA], f32, tag="kt")
            nc.sync.dma_start(kt0[:, :8, :], keys_d[0:P, 0:8, :])

            for j in range(NTILES):
                E = s_exts[j]
                qp = qps[j]

                # chunk schedule: geometric ramp on tile 0 so DVE starts
                # as soon as ~0.5MB has landed and never starves early.
                chunks = []
                c0 = 0
                if j == 0:
                    for ch in (8, 16, 26):
                        chunks.append((c0, ch))
                        c0 += ch
                while c0 < E:
                    ch = min(CH, E - c0)
                    chunks.append((c0, ch))
                    c0 += ch

                att = smallp.tile([P, E], f32, tag="att")
                for c0, ch in chunks:
                    if j == 0 and c0 == 0:
                        kt = kt0  # prefetched above
                    else:
                        kt = keysp.tile([P, CH, KDA], f32, tag="kt")
                        nc.sync.dma_start(
                            kt[:, :ch, :],
                            keys_d[j * P : (j + 1) * P, c0 : c0 + ch, :],
                        )
                    for s in range(ch):
                        # masked score: (keys_aug_s * 1.0) * qp_aug,
                        # accum_out = sum -> att[:, s]  (includes penalty)
                        # (scalar_tensor_tensor lowers to the native
                        # TensorScalarPtr opcode; tensor_tensor_reduce's
                        # custom ISA opcode crashes the runtime here.)
                        scr = scrp.tile([P, KDA], f32, tag="scr")
                        nc.vector.scalar_tensor_tensor(
                            scr[:],
                            kt[:, s, :],
                            1.0,
                            qp[:],
                            op0=mybir.AluOpType.mult,
                            op1=mybir.AluOpType.mult,
                            accum_out=att[:, c0 + s : c0 + s + 1],
                        )

                # no max-subtraction: |att| <= ~60 here (qp,keys ~ N(0,1),
                # softmax is shift-invariant, exp stays finite in f32);
                # seq_len==0 rows would give 0/0 but the host overwrites them.
                e_t = smallp.tile([P, E], f32, tag="e")
                ssum = smallp.tile([P, 1], f32, tag="ssum")
                nc.scalar.activation(
                    e_t[:],
                    att[:],
                    mybir.ActivationFunctionType.Exp,
                    bias=0.0,
                    scale=1.0,
                    accum_out=ssum[:],
                )
                rec = smallp.tile([P, 1], f32, tag="rec")
                nc.vector.reciprocal(rec[:], ssum[:])
                o_t = smallp.tile([P, E], f32, tag="o")
                # final scale on the (otherwise idle) ACT engine
                nc.scalar.mul(o_t[:], e_t[:], rec[:])
                # out via SWDGE (gpsimd) so the Sync queue carries only
                # keys chunks -- a keys issue never waits behind an out issue
                nc.gpsimd.dma_start(out_d[j * P : (j + 1) * P, 0:E], o_t[:])
    nc.compile()
    return nc


def _prep(query, keys, seq_len, w):
    query = np.ascontiguousarray(np.asarray(query), dtype=np.float32)
    keys = np.ascontiguousarray(np.asarray(keys), dtype=np.float32)
    w = np.ascontiguousarray(np.asarray(w), dtype=np.float32)
    lens = np.asarray(seq_len).reshape(B).astype(np.int64)

    order = np.argsort(-lens, kind="stable")
    gp = NCORES * P  # batches per tile slot across all cores
    slot_max = [int(lens[order[j * gp : (j + 1) * gp]].max()) for j in range(NTILES)]
    s_exts = tuple(min(S, max(1, m)) for m in slot_max)

    perms = []
    for c in range(NCORES):
        perms.append(
            np.concatenate(
                [order[j * gp : (j + 1) * gp][c::NCORES] for j in range(NTILES)]
            )
        )

    wT = np.ascontiguousarray(w.T)
    arange_s = np.arange(S, dtype=np.int64)[None, :]
    in_maps = []
    for c in range(NCORES):
        pc = perms[c]
        qT = query[pc, 0, :].reshape(NTILES, P, QD).transpose(2, 0, 1)
        qw = np.empty((QD, NTILES, P + KD), dtype=np.float32)
        qw[:, :, :P] = qT
        qw[:, :, P:] = wT[:, None, :]
        keys_aug = np.empty((PB, S, KDA), dtype=np.float32)
        keys_aug[:, :, :KD] = keys[pc]
        keys_aug[:, :, KD] = np.where(
            arange_s < lens[pc][:, None], 0.0, np.float32(NEG_INF)
        )
        in_maps.append({"keys": keys_aug, "qw": qw})
    return lens, s_exts, perms, in_maps


def kernel(query, keys, seq_len, w):
    global LAST_RESULTS
    lens, s_exts, perms, in_maps = _prep(query, keys, seq_len, w)

    nc = _nc_cache.get(s_exts)
    if nc is None:
        nc = _build(s_exts)
        _nc_cache[s_exts] = nc

    res = run_bass_kernel_spmd(nc, in_maps, core_ids=list(range(NCORES)))
    LAST_RESULTS = res

    out = np.zeros((B, S), dtype=np.float32)
    for c in range(NCORES):
        dev = np.asarray(res.results[c]["out"])
        pc = perms[c]
        for j in range(NTILES):
            E = s_exts[j]
            rows = pc[j * P : (j + 1) * P]
            out[rows, :E] = dev[j * P : (j + 1) * P, :E]
    out[lens == 0, :] = np.float32(1.0 / S)
    return out



# revision 3
# speedup vs baseline: 1.4383x; 1.4383x over previous
"""Trainium2 Bass kernel for masked attention softmax (ragged sequences).

Reference (per batch b):
    qp[k]  = sum_q query[b,0,q] * w[k,q]
    att[s] = sum_k qp[k] * keys[b,s,k]
    out[b] = softmax(where(s < seq_len[b], att, -inf))

Strategy (v2 -- TensorEngine matvec, fp16 stream):
  - Sort batches by len desc, deal round-robin to the 8 cores so slot j
    has (nearly) the same length on every core; slot extent ext_j =
    max len over the 8 cores at slot j is baked into the one compiled
    program (cache key = the ext tuple).
  - Host packs keys TRANSPOSED per slot: kt[k=128, sum(ext) cols] fp16.
    DMA is therefore exactly sum(ext)*128*2 bytes/core (~13 MB, half of
    the fp32 baseline, ragged -- no rectangle waste).
  - Per slot j the PE computes the batched matvec directly:
        matmul(out=score[0:ext, col_j], lhsT=kt[:, off:off+ext],
               rhs=qpT[:, j])
    i.e. keys is the *stationary* operand (LDWEIGHTS streams ~1 col/cyc,
    2 cols/cyc with FWL on full-128 loads), the per-batch projected
    query is a single moving column.  Scores land in PSUM transposed:
    [s on partitions, batch on free].  ext>128 uses a second matmul
    into a "hi" PSUM tile.
  - Softmax on the transposed layout: ACT exp (PSUM->SBUF fp32, no
    max-subtraction needed: |score| <= ~60 fits fp32 exp), DVE multiply
    by a host-built 0/1 mask (also kills garbage rows: stale PSUM and
    s >= len), then the sum over s (= partition axis) is done by the PE
    itself with a ones-column matmul; DVE reciprocal; rec broadcast to
    all partitions via a PE outer-product; DVE scale.  Group-g softmax
    work is emitted 1-2 groups late so PE never stalls on ACT/DVE.
  - Output [4, 200, 128] = [group][s][slot col]; host transposes,
    zeroes s >= len, fills len==0 rows with 1/S, scatters by the
    inverse permutation.  fp16 quantization of keys/qp gives max rel
    err ~4.5e-3 (measured vs fp64 reference; gate is 2e-2).
"""

import sys

import numpy as np

sys.path.insert(0, "/opt/trn_rl_repo")

import concourse.bass as bass
import concourse.tile as tile
from concourse import bacc, mybir
from concourse.bass_utils import run_bass_kernel_spmd


def _install_trace_shims():
    """The agent image lacks ``antenv.axon_hooks``, so trace=True silently
    degrades.  Recreate the module and register the ctypes NTFF hook from
    trn_agent_boot; also make artifact upload failure non-fatal."""
    try:
        import types

        import antenv
        from concourse import bass_utils as _bu

        if "antenv.axon_hooks" not in sys.modules:
            mod = types.ModuleType("antenv.axon_hooks")
            mod._hook = None
            mod.set_axon_ntff_profile_hook = lambda h: setattr(mod, "_hook", h)
            mod.get_axon_ntff_profile_hook = lambda: mod._hook
            sys.modules["antenv.axon_hooks"] = mod
            antenv.axon_hooks = mod
            from trn_agent_boot.trn_boot import _ntff_profile_via_ctypes

            mod.set_axon_ntff_profile_hook(
                _ntff_profile_via_ctypes("/opt/axon/libaxon_pjrt.so")
            )

        _orig_upload = _bu.upload_artifacts

        def _safe_upload(tmpdir):
            try:
                return _orig_upload(tmpdir)
            except Exception:
                return "local://" + str(tmpdir)

        _bu.upload_artifacts = _safe_upload
    except Exception:
        pass


_install_trace_shims()

B, S, KD, QD = 4096, 200, 128, 128
NCORES = 8
P = 128
NSLOTS = B // NCORES          # 512 slots (batches) per core
NGROUPS = NSLOTS // P         # 4 groups of 128 slots

LAST_RESULTS = None
_nc_cache = {}


def _plan_chunks(exts):
    """Split slots into DMA chunks (whole slots, ~col targets with a
    geometric ramp so the PE starts early)."""
    targets = [512, 1024, 2048]
    chunks = []  # (j0, j1, c0, c1)
    j, c, ti = 0, 0, 0
    n = len(exts)
    while j < n:
        tgt = targets[ti] if ti < len(targets) else 4096
        j0, c0 = j, c
        while j < n and c - c0 < tgt:
            c += exts[j]
            j += 1
        chunks.append((j0, j, c0, c))
        ti += 1
    return chunks


def _build(exts):
    exts = list(exts)
    f32, f16 = mybir.dt.float32, mybir.dt.float16
    Exp = mybir.ActivationFunctionType.Exp

    los = [min(e, P) for e in exts]
    his = [e - lo for e, lo in zip(exts, los)]
    offs = np.concatenate([[0], np.cumsum(exts)]).astype(int)
    TOT = int(offs[-1])
    ghi = [max(his[g * P : (g + 1) * P]) for g in range(NGROUPS)]
    chunks = _plan_chunks(exts)
    slot_chunk = {}
    for ci, (j0, j1, c0, c1) in enumerate(chunks):
        for j in range(j0, j1):
            slot_chunk[j] = (ci, c0)

    nc = bacc.Bacc("TRN2", target_bir_lowering=False, debug=False)
    kt_d = nc.dram_tensor("kt", [P, TOT], f16, kind="ExternalInput")
    qw_d = nc.dram_tensor("qw", [P, KD + NSLOTS], f16, kind="ExternalInput")
    mk_d = nc.dram_tensor("mk", [P, 2 * NSLOTS], f32, kind="ExternalInput")
    out_d = nc.dram_tensor("out", [NGROUPS, S, P], f32, kind="ExternalOutput")

    with tile.TileContext(nc) as tc:
        with (
            tc.tile_pool(name="ktp", bufs=1) as ktp,
            tc.tile_pool(name="small", bufs=1) as smallp,
            tc.tile_pool(name="psum", bufs=1, space=bass.MemorySpace.PSUM) as psump,
        ):
            # --- small inputs; qw first so qp is ready before chunk0 lands
            qw = smallp.tile([P, KD + NSLOTS], f16, tag="qw")
            nc.sync.dma_start(qw[:], qw_d[:])
            mk = smallp.tile([P, 2 * NSLOTS], f32, tag="mk")
            nc.gpsimd.dma_start(mk[:], mk_d[:])

            # qpT[k, j] = sum_q wT[q, k] * qT[q, j]
            qp_ps = psump.tile([P, NSLOTS], f32, tag="qp_ps")
            nc.tensor.matmul(
                qp_ps[:], qw[:, 0:KD], qw[:, KD : KD + NSLOTS],
                start=True, stop=True,
            )
            qpt = smallp.tile([P, NSLOTS], f16, tag="qpt")
            nc.scalar.copy(qpt[:], qp_ps[:])

            ones_col = smallp.tile([P, 1], f32, tag="ones_col")
            nc.vector.memset(ones_col[:], 1.0)
            ones_row = smallp.tile([1, P], f32, tag="ones_row")
            nc.vector.memset(ones_row[:], 1.0)

            # score PSUM tiles (2-deep ping-pong x lo/hi); zeroed once so
            # stale reads are always finite
            sc_lo = [psump.tile([P, P], f32, tag=f"sclo{i}", name=f"sclo{i}") for i in range(2)]
            sc_hi = [psump.tile([P, P], f32, tag=f"schi{i}", name=f"schi{i}") for i in range(2)]
            for t in sc_lo + sc_hi:
                nc.vector.memset(t[:], 0.0)

            ssum_ps = psump.tile([1, NSLOTS], f32, tag="ssum")
            recb_ps = psump.tile([P, NSLOTS], f32, tag="recb")
            rec_t = smallp.tile([1, NSLOTS], f32, tag="rec")

            em_lo = [smallp.tile([P, P], f32, tag=f"emlo{i}", name=f"emlo{i}") for i in range(2)]
            em_hi = [smallp.tile([P, P], f32, tag=f"emhi{i}", name=f"emhi{i}") for i in range(2)]
            o_lo = [smallp.tile([P, P], f32, tag=f"olo{i}", name=f"olo{i}") for i in range(2)]
            o_hi = [smallp.tile([P, P], f32, tag=f"ohi{i}", name=f"ohi{i}") for i in range(2)]

            # --- stream in all keysT chunks (HWDGE, in order)
            ck_tiles = []
            for ci, (j0, j1, c0, c1) in enumerate(chunks):
                t = ktp.tile([P, c1 - c0], f16, tag=f"ck{ci}", name=f"ck{ci}")
                nc.sync.dma_start(t[:], kt_d[:, c0:c1])
                ck_tiles.append(t)

            def emit_group_mms(g):
                lo_t, hi_t = sc_lo[g % 2], sc_hi[g % 2]
                for j in range(g * P, (g + 1) * P):
                    col = j - g * P
                    ci, c0 = slot_chunk[j]
                    o = int(offs[j]) - c0
                    ck = ck_tiles[ci]
                    nc.tensor.matmul(
                        lo_t[0 : los[j], col : col + 1],
                        ck[:, o : o + los[j]],
                        qpt[:, j : j + 1],
                        start=True, stop=True,
                    )
                    if his[j] > 0:
                        nc.tensor.matmul(
                            hi_t[0 : his[j], col : col + 1],
                            ck[:, o + P : o + P + his[j]],
                            qpt[:, j : j + 1],
                            start=True, stop=True,
                        )

            def emit_softmax_a(g):
                # exp, mask, and the PE partition-sum for group g
                lo_t, hi_t = sc_lo[g % 2], sc_hi[g % 2]
                el, eh = em_lo[g % 2], em_hi[g % 2]
                gh = ghi[g]
                gcols = slice(g * P, (g + 1) * P)
                nc.scalar.activation(el[:], lo_t[:], Exp, bias=0.0, scale=1.0)
                nc.vector.tensor_mul(el[:], el[:], mk[:, gcols])
                if gh > 0:
                    nc.scalar.activation(eh[0:gh, :], hi_t[0:gh, :], Exp,
                                         bias=0.0, scale=1.0)
                    nc.vector.tensor_mul(
                        eh[0:gh, :], eh[0:gh, :],
                        mk[0:gh, NSLOTS + g * P : NSLOTS + (g + 1) * P],
                    )
                nc.tensor.matmul(
                    ssum_ps[0:1, gcols], ones_col[0:P, 0:1], el[:],
                    start=True, stop=(gh == 0),
                )
                if gh > 0:
                    nc.tensor.matmul(
                        ssum_ps[0:1, gcols], ones_col[0:gh, 0:1], eh[0:gh, :],
                        start=False, stop=True,
                    )

            def emit_softmax_b(g):
                # reciprocal, broadcast, scale, store for group g
                el, eh = em_lo[g % 2], em_hi[g % 2]
                ol, oh = o_lo[g % 2], o_hi[g % 2]
                gh = ghi[g]
                gcols = slice(g * P, (g + 1) * P)
                nc.vector.reciprocal(rec_t[0:1, gcols], ssum_ps[0:1, gcols])
                nc.tensor.matmul(
                    recb_ps[:, gcols], ones_row[0:1, 0:P], rec_t[0:1, gcols],
                    start=True, stop=True,
                )
                nc.vector.tensor_mul(ol[:], el[:], recb_ps[:, gcols])
                nc.gpsimd.dma_start(out_d[g, 0:P, :], ol[:])
                if gh > 0:
                    nc.vector.tensor_mul(
                        oh[0:gh, :], eh[0:gh, :], recb_ps[0:gh, gcols]
                    )
                    nc.gpsimd.dma_start(out_d[g, P : P + gh, :], oh[0:gh, :])

            # pipeline: softmax for group g is emitted 1-2 groups late so
            # its PE ops (ssum / rec-broadcast) never stall the MM stream
            emit_group_mms(0)
            emit_group_mms(1)
            emit_softmax_a(0)
            emit_group_mms(2)
            emit_softmax_b(0)
            emit_softmax_a(1)
            emit_group_mms(3)
            emit_softmax_b(1)
            emit_softmax_a(2)
            emit_softmax_b(2)
            emit_softmax_a(3)
            emit_softmax_b(3)

    nc.compile()
    return nc


def _prep(query, keys, seq_len, w):
    query = np.asarray(query)
    keys = np.asarray(keys)
    w = np.asarray(w)
    lens = np.asarray(seq_len).reshape(B).astype(np.int64)

    order = np.argsort(-lens, kind="stable")
    exts = np.maximum(1, np.minimum(S, lens[order[0::NCORES]])).astype(int)
    offs = np.concatenate([[0], np.cumsum(exts)]).astype(int)
    TOT = int(offs[-1])

    keys16 = np.ascontiguousarray(keys, dtype=np.float16)
    q16 = np.ascontiguousarray(query[:, 0, :], dtype=np.float16)
    wT16 = np.ascontiguousarray(w.T, dtype=np.float16)

    sv = np.arange(P)[:, None]
    in_maps = []
    perms = []
    for c in range(NCORES):
        idx = order[c::NCORES]
        perms.append(idx)
        l_c = lens[idx]

        kt = np.zeros((P, TOT), dtype=np.float16)
        for j in range(NSLOTS):
            e = int(exts[j])
            kt[:, offs[j] : offs[j] + e] = keys16[idx[j], :e, :].T

        qw = np.zeros((P, KD + NSLOTS), dtype=np.float16)
        qw[:, :KD] = wT16
        qw[:, KD:] = q16[idx].T

        mk = np.zeros((P, 2 * NSLOTS), dtype=np.float32)
        mk[:, :NSLOTS] = sv < l_c[None, :]
        mk[:, NSLOTS:] = (P + sv) < l_c[None, :]

        in_maps.append({"kt": kt, "qw": qw, "mk": mk})
    return lens, exts, perms, in_maps


def kernel(query, keys, seq_len, w):
    global LAST_RESULTS
    lens, exts, perms, in_maps = _prep(query, keys, seq_len, w)

    key = tuple(int(e) for e in exts)
    nc = _nc_cache.get(key)
    if nc is None:
        nc = _build(key)
        _nc_cache[key] = nc

    res = run_bass_kernel_spmd(nc, in_maps, core_ids=list(range(NCORES)))
    LAST_RESULTS = res

    out = np.zeros((B, S), dtype=np.float32)
    sv = np.arange(S)[None, :]
    for c in range(NCORES):
        dev = np.asarray(res.results[c]["out"])  # [NGROUPS, S, P]
        arr = dev.transpose(0, 2, 1).reshape(NSLOTS, S)
        l_c = lens[perms[c]]
        arr = np.where(sv < l_c[:, None], arr, 0.0).astype(np.float32)
        arr[l_c == 0] = np.float32(1.0 / S)
        out[perms[c]] = arr
    return out


# revision 4
# speedup vs baseline: 1.5358x; 1.0678x over previous
"""Trainium2 Bass kernel for masked attention softmax (ragged sequences).

Reference (per batch b):
    qp[k]  = sum_q query[b,0,q] * w[k,q]
    att[s] = sum_k qp[k] * keys[b,s,k]
    out[b] = softmax(where(s < seq_len[b], att, -inf))

Strategy (v3 -- hybrid PE matvec + DVE STT, fp16 stream):
  - Sort batches by len desc, deal round-robin to the 8 cores; slot
    extent ext_j = max len over the 8 cores at slot j is baked into the
    single compiled program (cache key = ext tuple).
  - Work split across engines (measured rates: PE ~107ns per
    weight-swap LDW+MM pair regardless of cols; DVE STT ~130-200ns per
    s-position per 128-batch tile):
      * DVE tiles = slots 0..127 (longest, E0~199) and 384..511
        (shortest, E1~50): per position one scalar_tensor_tensor
        (keys_aug * 1.0) * qp_aug with accum -> att, fp16 inputs.
        Mask folded into keys element 128 (0 or -3e4; elem 129 pad).
      * PE slots = 128..383: per slot matmul(out=score[0:ext, col],
        lhsT=ktT[:, off:off+ext], rhs=qpT[:, j]) -- keys stationary,
        query-projection moving; scores land [s-part, batch-col] in
        PSUM.
  - PE-side masking without mask tiles: host REPLACES key columns at
    s in [len, ext) by qp_b * (-3e4/||qp_b||^2) so their score is -3e4
    (exp -> 0 exactly); score PSUM tiles are single-use and zeroed once
    at start so never-written rows give exp(0)=1, corrected by a
    host-shipped per-slot count subtracted from the PE ones-matmul
    column sum.  Reciprocal via reciprocal_approx_fast, broadcast over
    partitions via a PE outer product, DVE scale, store.
  - DVE-side softmax is per-partition (batch on partitions): ACT exp
    with accum_out, DVE reciprocal [P,1], ACT scale.
  - All keys chunks (both layouts) stream on the sync HWDGE queue,
    interleaved by consumption deadline; outputs ride the same queue at
    the end; tiny inputs on SWDGE.
  - fp16 keys/qp quantization: max rel err ~4.5e-3 measured vs fp64
    (gate 2e-2).  exp stays fp32 (scores up to ~60 overflow fp16).
"""

import sys

import numpy as np

sys.path.insert(0, "/opt/trn_rl_repo")

import concourse.bass as bass
import concourse.tile as tile
from concourse import bacc, mybir
from concourse.bass_utils import run_bass_kernel_spmd


def _install_trace_shims():
    """The agent image lacks ``antenv.axon_hooks``, so trace=True silently
    degrades.  Recreate the module and register the ctypes NTFF hook from
    trn_agent_boot; also make artifact upload failure non-fatal."""
    try:
        import types

        import antenv
        from concourse import bass_utils as _bu

        if "antenv.axon_hooks" not in sys.modules:
            mod = types.ModuleType("antenv.axon_hooks")
            mod._hook = None
            mod.set_axon_ntff_profile_hook = lambda h: setattr(mod, "_hook", h)
            mod.get_axon_ntff_profile_hook = lambda: mod._hook
            sys.modules["antenv.axon_hooks"] = mod
            antenv.axon_hooks = mod
            from trn_agent_boot.trn_boot import _ntff_profile_via_ctypes

            mod.set_axon_ntff_profile_hook(
                _ntff_profile_via_ctypes("/opt/axon/libaxon_pjrt.so")
            )

        _orig_upload = _bu.upload_artifacts

        def _safe_upload(tmpdir):
            try:
                return _orig_upload(tmpdir)
            except Exception:
                return "local://" + str(tmpdir)

        _bu.upload_artifacts = _safe_upload
    except Exception:
        pass


_install_trace_shims()

B, S, KD, QD = 4096, 200, 128, 128
NCORES = 8
P = 128
NSLOTS = B // NCORES          # 512 slots (batches) per core
KAUG = 130                    # keys + mask elem + pad elem (4B-aligned rows)
BIGNEG = -30000.0

# slot ranges per engine
DVE_T0 = (0, 128)             # longest slots
DVE_T1 = (384, 512)           # shortest slots
PE_LO, PE_HI = 128, 384       # PE slots, two groups of 128

LAST_RESULTS = None
_nc_cache = {}


def _chunk_sizes(total, ramp, steady):
    sizes, c = [], 0
    for r in ramp:
        if c >= total:
            break
        s = min(r, total - c)
        sizes.append(s)
        c += s
    while c < total:
        s = min(steady, total - c)
        sizes.append(s)
        c += s
    return sizes


def _build(exts):
    exts = list(exts)
    f32, f16 = mybir.dt.float32, mybir.dt.float16
    Exp = mybir.ActivationFunctionType.Exp
    E0 = exts[DVE_T0[0]]
    E1 = exts[DVE_T1[0]]

    pe_slots = list(range(PE_LO, PE_HI))
    los = {j: min(exts[j], P) for j in pe_slots}
    his = {j: exts[j] - los[j] for j in pe_slots}
    # packed kt offsets for PE slots
    offs = {}
    c = 0
    for j in pe_slots:
        offs[j] = c
        c += exts[j]
    TOTP = c
    gh = [max(his[j] for j in range(PE_LO + g * P, PE_LO + (g + 1) * P))
          for g in range(2)]

    nc = bacc.Bacc("TRN2", target_bir_lowering=False, debug=False)
    kv_d = nc.dram_tensor("kv", [P, E0 + E1, KAUG], f16, kind="ExternalInput")
    kt_d = nc.dram_tensor("kt", [P, TOTP], f16, kind="ExternalInput")
    qw_d = nc.dram_tensor("qw", [P, KD + NSLOTS], f16, kind="ExternalInput")
    cn_d = nc.dram_tensor("cn", [1, 2 * P], f32, kind="ExternalInput")
    op_d = nc.dram_tensor("op", [2, S, P], f32, kind="ExternalOutput")
    ov_d = nc.dram_tensor("ov", [2, P, S], f32, kind="ExternalOutput")

    # --- chunk plans ----------------------------------------------------
    # DVE kv chunks (positions): tile0 then tile1
    kv0_sizes = _chunk_sizes(E0, [16, 24, 32, 40], 48)
    kv1_sizes = _chunk_sizes(E1, [26], 24)
    # PE kt chunks (cols), aligned to slot boundaries
    kt_targets = _chunk_sizes(TOTP, [512, 1024, 2048, 3072], 4096)
    kt_chunks = []  # (j0, j1, c0, c1)
    ji = 0
    c = 0
    for tgt in kt_targets:
        j0, c0 = ji, c
        while ji < len(pe_slots) and c - c0 < tgt:
            c += exts[pe_slots[ji]]
            ji += 1
        kt_chunks.append((pe_slots[j0], pe_slots[ji - 1] + 1, c0, c))
        if ji >= len(pe_slots):
            break
    slot_chunk = {}
    for ci, (j0, j1, c0, c1) in enumerate(kt_chunks):
        for j in range(j0, j1):
            slot_chunk[j] = (ci, c0)

    with tile.TileContext(nc) as tc:
        with (
            tc.tile_pool(name="ktp", bufs=1) as ktp,
            tc.tile_pool(name="kvp", bufs=1) as kvp,
            tc.tile_pool(name="small", bufs=1) as smallp,
            tc.tile_pool(name="scr", bufs=16) as scrp,
            tc.tile_pool(name="psum", bufs=1, space=bass.MemorySpace.PSUM) as psump,
        ):
            # --- tiny inputs
            qw = smallp.tile([P, KD + NSLOTS], f16, tag="qw")
            nc.sync.dma_start(qw[:], qw_d[:])
            cn = smallp.tile([1, 2 * P], f32, tag="cn")
            nc.gpsimd.dma_start(cn[:], cn_d[:])

            # --- keys chunk DMAs, interleaved by deadline on sync queue
            kv_tiles = []   # (tile_ap, pos0, npos) in kv_d position space
            kt_tiles = []
            emit_plan = []  # ("kv"/"kt", index into plans)
            # deadline interleave: kv0 ramp first, then alternate
            kv_pos = []
            p0 = 0
            for s_ in kv0_sizes:
                kv_pos.append((p0, s_))
                p0 += s_
            for s_ in kv1_sizes:
                kv_pos.append((p0, s_))
                p0 += s_
            nkv, nkt = len(kv_pos), len(kt_chunks)
            # simple merge: kv0 first two, then alternate kt/kv
            order = []
            ia = ib = 0
            # DVE rate ~0.13us/pos, PE ~0.11us/slot-pair: interleave 1:1
            while ia < nkv or ib < nkt:
                if ia < nkv:
                    order.append(("kv", ia)); ia += 1
                if ib < nkt:
                    order.append(("kt", ib)); ib += 1
            for kind, i in order:
                if kind == "kv":
                    pos0, npos = kv_pos[i]
                    t = kvp.tile([P, npos, KAUG], f16, tag=f"kv{i}",
                                 name=f"kv{i}")
                    nc.sync.dma_start(t[:], kv_d[:, pos0 : pos0 + npos, :])
                    kv_tiles.append((t, pos0, npos))
                else:
                    j0, j1, c0, c1 = kt_chunks[i]
                    t = ktp.tile([P, c1 - c0], f16, tag=f"kt{i}",
                                 name=f"kt{i}")
                    nc.sync.dma_start(t[:], kt_d[:, c0:c1])
                    kt_tiles.append(t)

            # --- qp projections (PE) -------------------------------------
            # qpT[k, j] for PE slots:   lhsT=wT [q,k], rhs=qT cols
            qpt_ps = psump.tile([P, 2 * P], f32, tag="qpt_ps")
            nc.tensor.matmul(
                qpt_ps[:], qw[:, 0:KD], qw[:, KD + PE_LO : KD + PE_HI],
                start=True, stop=True,
            )
            # qp[b, k] for DVE tiles:  lhsT=qT cols [q,b], rhs=wT [q,k]
            qpb_ps = [psump.tile([P, P], f32, tag=f"qpb{t}", name=f"qpb{t}")
                      for t in range(2)]
            for t, (s0, s1) in enumerate((DVE_T0, DVE_T1)):
                nc.tensor.matmul(
                    qpb_ps[t][:], qw[:, KD + s0 : KD + s1], qw[:, 0:KD],
                    start=True, stop=True,
                )

            qpt = smallp.tile([P, 2 * P], f16, tag="qpt")
            nc.scalar.copy(qpt[:], qpt_ps[:])
            qp_aug = [smallp.tile([P, KAUG], f16, tag=f"qpa{t}", name=f"qpa{t}")
                      for t in range(2)]
            for t in range(2):
                nc.scalar.copy(qp_aug[t][:, 0:KD], qpb_ps[t][:])
                nc.vector.memset(qp_aug[t][:, KD : KD + 1], 1.0)
                nc.vector.memset(qp_aug[t][:, KD + 1 : KAUG], 0.0)

            ones_col = smallp.tile([P, 1], f32, tag="ones_col")
            nc.vector.memset(ones_col[:], 1.0)
            ones_row = smallp.tile([1, P], f32, tag="ones_row")
            nc.vector.memset(ones_row[:], 1.0)

            # --- PSUM score tiles (single-use per group, zeroed once)
            sc_lo = [psump.tile([P, P], f32, tag=f"sclo{i}", name=f"sclo{i}")
                     for i in range(2)]
            sc_hi = psump.tile([P, P], f32, tag="schi")
            for t in sc_lo:
                nc.vector.memset(t[:], 0.0)
            nc.vector.memset(sc_hi[:], 0.0)

            ssum_ps = psump.tile([1, 2 * P], f32, tag="ssum")
            recb_ps = psump.tile([P, 2 * P], f32, tag="recb")
            ssc = smallp.tile([1, 2 * P], f32, tag="ssc")
            rec_t = smallp.tile([1, 2 * P], f32, tag="rec")

            em_lo = [smallp.tile([P, P], f32, tag=f"emlo{i}", name=f"emlo{i}")
                     for i in range(2)]
            em_hi = smallp.tile([P, P], f32, tag="emhi")
            o_lo = [smallp.tile([P, P], f32, tag=f"olo{i}", name=f"olo{i}")
                    for i in range(2)]
            o_hi = smallp.tile([P, P], f32, tag="ohi")

            # --- DVE side: att accumulators + per-tile softmax ----------
            att = [smallp.tile([P, E0], f32, tag="att0", name="att0"),
                   smallp.tile([P, E1], f32, tag="att1", name="att1")]
            tile_rng = {0: (0, E0), 1: (E0, E0 + E1)}

            def emit_dve_stts(t):
                base, end = tile_rng[t]
                for (ck, pos0, npos) in kv_tiles:
                    lo = max(pos0, base)
                    hi = min(pos0 + npos, end)
                    for p in range(lo, hi):
                        i = p - pos0
                        s = p - base
                        scr = scrp.tile([P, KAUG], f16, tag="scr")
                        nc.vector.scalar_tensor_tensor(
                            scr[:],
                            ck[:, i, :],
                            1.0,
                            qp_aug[t][:],
                            op0=mybir.AluOpType.mult,
                            op1=mybir.AluOpType.mult,
                            accum_out=att[t][:, s : s + 1],
                        )

            def emit_dve_softmax(t):
                E = E0 if t == 0 else E1
                e_t = smallp.tile([P, E], f32, tag=f"e{t}", name=f"e{t}")
                ssumv = smallp.tile([P, 1], f32, tag=f"ssv{t}", name=f"ssv{t}")
                nc.scalar.activation(
                    e_t[:], att[t][:], Exp, bias=0.0, scale=1.0,
                    accum_out=ssumv[:],
                )
                recv = smallp.tile([P, 1], f32, tag=f"rcv{t}", name=f"rcv{t}")
                nc.vector.reciprocal(recv[:], ssumv[:])
                o_t = smallp.tile([P, E], f32, tag=f"ov{t}", name=f"ovt{t}")
                nc.scalar.mul(o_t[:], e_t[:], recv[:])
                nc.sync.dma_start(ov_d[t, :, 0:E], o_t[:])

            # --- PE side ------------------------------------------------
            def emit_pe_mms(g):
                lo_t = sc_lo[g]
                for j in range(PE_LO + g * P, PE_LO + (g + 1) * P):
                    col = j - (PE_LO + g * P)
                    ci, c0 = slot_chunk[j]
                    o = offs[j] - c0
                    ck = kt_tiles[ci]
                    nc.tensor.matmul(
                        lo_t[0 : los[j], col : col + 1],
                        ck[:, o : o + los[j]],
                        qpt[:, j - PE_LO : j - PE_LO + 1],
                        start=True, stop=True,
                    )
                    if his[j] > 0:
                        nc.tensor.matmul(
                            sc_hi[0 : his[j], col : col + 1],
                            ck[:, o + P : o + P + his[j]],
                            qpt[:, j - PE_LO : j - PE_LO + 1],
                            start=True, stop=True,
                        )

            def emit_pe_softmax_a(g):
                lo_t = sc_lo[g]
                el = em_lo[g]
                gcols = slice(g * P, (g + 1) * P)
                nc.scalar.activation(el[:], lo_t[:], Exp, bias=0.0, scale=1.0)
                if gh[g] > 0:
                    nc.scalar.activation(em_hi[0 : gh[g], :],
                                         sc_hi[0 : gh[g], :],
                                         Exp, bias=0.0, scale=1.0)
                nc.tensor.matmul(
                    ssum_ps[0:1, gcols], ones_col[0:P, 0:1], el[:],
                    start=True, stop=(gh[g] == 0),
                )
                if gh[g] > 0:
                    nc.tensor.matmul(
                        ssum_ps[0:1, gcols], ones_col[0 : gh[g], 0:1],
                        em_hi[0 : gh[g], :],
                        start=False, stop=True,
                    )

            def emit_pe_softmax_b(g):
                el = em_lo[g]
                ol = o_lo[g]
                gcols = slice(g * P, (g + 1) * P)
                # subtract count of never-written rows (exp(0)=1 each)
                nc.vector.tensor_sub(ssc[0:1, gcols], ssum_ps[0:1, gcols],
                                     cn[0:1, gcols])
                nc.vector.reciprocal_approx_fast(rec_t[0:1, gcols],
                                                 ssc[0:1, gcols])
                nc.tensor.matmul(
                    recb_ps[:, gcols], ones_row[0:1, 0:P], rec_t[0:1, gcols],
                    start=True, stop=True,
                )
                nc.vector.tensor_mul(ol[:], el[:], recb_ps[:, gcols])
                nc.sync.dma_start(op_d[g, 0:P, :], ol[:])
                if gh[g] > 0:
                    nc.vector.tensor_mul(o_hi[0 : gh[g], :],
                                         em_hi[0 : gh[g], :],
                                         recb_ps[0 : gh[g], gcols])
                    nc.sync.dma_start(op_d[g, P : P + gh[g], :],
                                      o_hi[0 : gh[g], :])

            # --- emission schedule --------------------------------------
            # PE queue: qp MMs, then group MMs, then deferred reductions.
            # DVE queue: t0 STTs, (pe g0 reductions), t1 STTs, tails.
            emit_pe_mms(0)
            emit_dve_stts(0)
            emit_pe_mms(1)
            emit_pe_softmax_a(0)
            emit_pe_softmax_b(0)
            emit_dve_softmax(0)
            emit_dve_stts(1)
            emit_pe_softmax_a(1)
            emit_pe_softmax_b(1)
            emit_dve_softmax(1)

    nc.compile()
    return nc


def _prep(query, keys, seq_len, w):
    query = np.asarray(query)
    keys = np.asarray(keys)
    w = np.asarray(w)
    lens = np.asarray(seq_len).reshape(B).astype(np.int64)

    order = np.argsort(-lens, kind="stable")
    exts = np.maximum(1, np.minimum(S, lens[order[0::NCORES]])).astype(int)
    E0 = int(exts[DVE_T0[0]])
    E1 = int(exts[DVE_T1[0]])

    pe_slots = list(range(PE_LO, PE_HI))
    offs = {}
    c = 0
    for j in pe_slots:
        offs[j] = c
        c += int(exts[j])
    TOTP = c
    gh = [max(max(int(exts[j]) - P, 0)
              for j in range(PE_LO + g * P, PE_LO + (g + 1) * P))
          for g in range(2)]

    keys16 = np.ascontiguousarray(keys, dtype=np.float16)
    q32 = query[:, 0, :].astype(np.float32)
    qp32 = q32 @ w.astype(np.float32).T          # [B, KD] host qp for masks
    q16 = np.ascontiguousarray(q32, dtype=np.float16)
    wT16 = np.ascontiguousarray(w.T, dtype=np.float16)

    in_maps = []
    perms = []
    for cidx in range(NCORES):
        idx = order[cidx::NCORES]
        perms.append(idx)
        l_c = lens[idx]

        # DVE tiles: original layout + aug elements
        kv = np.zeros((P, E0 + E1, KAUG), dtype=np.float16)
        for t, ((s0, s1), E, base) in enumerate(
            ((DVE_T0, E0, 0), (DVE_T1, E1, E0))
        ):
            bidx = idx[s0:s1]
            kv[:, base : base + E, :KD] = keys16[bidx, :E, :]
            svE = np.arange(E)[None, :]
            kv[:, base : base + E, KD] = np.where(
                svE < l_c[s0:s1][:, None], 0.0, np.float16(BIGNEG)
            )

        # PE slots: transposed packed keys; cols s in [len, ext) replaced
        # by qp * (-3e4/||qp||^2) so their score is BIGNEG exactly
        kt = np.zeros((P, TOTP), dtype=np.float16)
        for j in pe_slots:
            e = int(exts[j])
            b = idx[j]
            o = offs[j]
            kt[:, o : o + e] = keys16[b, :e, :].T
            ln = int(l_c[j])
            if ln < e:
                qpb = qp32[b]
                mcol = (qpb * (BIGNEG / float(qpb @ qpb))).astype(np.float16)
                kt[:, o + ln : o + e] = mcol[:, None]

        qw = np.zeros((P, KD + NSLOTS), dtype=np.float16)
        qw[:, :KD] = wT16
        qw[:, KD:] = q16[idx].T

        # count of never-written score rows per PE slot (exp(0)=1 each)
        cnv = np.zeros((1, 2 * P), dtype=np.float32)
        for g in range(2):
            for j in range(PE_LO + g * P, PE_LO + (g + 1) * P):
                e = int(exts[j])
                lo_stale = P - min(e, P)
                hi_stale = gh[g] - max(e - P, 0) if gh[g] > 0 else 0
                cnv[0, g * P + (j - PE_LO - g * P)] = lo_stale + hi_stale

        in_maps.append({"kv": kv, "kt": kt, "qw": qw, "cn": cnv})
    return lens, exts, perms, in_maps


def kernel(query, keys, seq_len, w):
    global LAST_RESULTS
    lens, exts, perms, in_maps = _prep(query, keys, seq_len, w)

    key = tuple(int(e) for e in exts)
    nc = _nc_cache.get(key)
    if nc is None:
        nc = _build(key)
        _nc_cache[key] = nc

    res = run_bass_kernel_spmd(nc, in_maps, core_ids=list(range(NCORES)))
    LAST_RESULTS = res

    E0 = int(exts[DVE_T0[0]])
    E1 = int(exts[DVE_T1[0]])
    out = np.zeros((B, S), dtype=np.float32)
    sv = np.arange(S)[None, :]
    for c in range(NCORES):
        dev_p = np.asarray(res.results[c]["op"])   # [2, S, P]
        dev_v = np.asarray(res.results[c]["ov"])   # [2, P, S]
        idx = perms[c]
        l_c = lens[idx]

        full = np.zeros((NSLOTS, S), dtype=np.float32)
        full[DVE_T0[0] : DVE_T0[1], :] = dev_v[0]
        full[DVE_T1[0] : DVE_T1[1], :] = dev_v[1]
        full[PE_LO : PE_LO + P, :] = dev_p[0].T
        full[PE_LO + P : PE_HI, :] = dev_p[1].T

        arr = np.where(sv < l_c[:, None], full, 0.0).astype(np.float32)
        arr[l_c == 0] = np.float32(1.0 / S)
        out[idx] = arr
    return out


# revision 7
# speedup vs baseline: 1.8271x; 1.1897x over previous
"""Trainium2 Bass kernel for masked attention softmax (ragged sequences).

Reference (per batch b):
    qp[k]  = sum_q query[b,0,q] * w[k,q]
    att[s] = sum_k qp[k] * keys[b,s,k]
    out[b] = softmax(where(s < seq_len[b], att, -inf))

Strategy (v4 -- hybrid PE matvec + DVE STT, fp16 stream, tuned split):
  - Sort batches by len desc, deal round-robin to the 8 cores; slot
    extent ext_j = max len over the 8 cores at slot j is baked into the
    single compiled program (cache key = ext tuple).
  - Measured engine rates (HW): PE per-slot matvec ~1.2-1.5 ns/key-col
    + ~40ns/matmul; DVE scalar_tensor_tensor ~215 ns per s-position per
    128-batch tile REGARDLESS of dtype (fused accum blocks the 2x DVE
    perf mode; plain fp16 TT hits 136ns but cannot accumulate).  Both
    engines therefore cost ~1.5-1.7 ns per batch-position -> split the
    work: PE takes the SPLIT longest slots, DVE the rest as 128-batch
    tiles.  fp16 inputs halve HBM traffic at zero compute cost.
  - PE per slot j: matmul(out=score[0:ext, col], lhsT=ktT[:, off:],
    rhs=qpT[:, j]) -- keys stationary (exact ragged col count), query
    projection moving; scores land [s-part, batch-col] in PSUM; sum
    over s via a ones-column matmul on the PE itself; host-crafted
    "-3e4 dot" key columns implement masking; never-written score rows
    (zeroed PSUM) contribute exp(0)=1, removed by a host count row;
    reciprocal_approx_fast + PE outer-product broadcast + DVE scale.
  - DVE tiles: per position one STT (keys_aug*1.0)*qp_aug, fp16 in /
    fp32 out+accum; mask folded into keys element 128 (0 / -3e4),
    element 129 pads rows to 4B alignment.  ACT exp with accum_out,
    DVE reciprocal [P,1], ACT scale.
  - Keys chunks for both layouts stream on the sync HWDGE queue in
    consumption-deadline order (a naive interleave starves the PE).
  - fp16 keys/qp quantization: max rel err ~4.5e-3 vs fp64 (gate 2e-2);
    exp stays fp32 (scores up to ~60 overflow fp16).
"""

import sys

import numpy as np

sys.path.insert(0, "/opt/trn_rl_repo")

import concourse.bass as bass
import concourse.tile as tile
from concourse import bacc, mybir
from concourse.bass_utils import run_bass_kernel_spmd


def _install_trace_shims():
    """The agent image lacks ``antenv.axon_hooks``, so trace=True silently
    degrades.  Recreate the module and register the ctypes NTFF hook from
    trn_agent_boot; also make artifact upload failure non-fatal."""
    try:
        import types

        import antenv
        from concourse import bass_utils as _bu

        if "antenv.axon_hooks" not in sys.modules:
            mod = types.ModuleType("antenv.axon_hooks")
            mod._hook = None
            mod.set_axon_ntff_profile_hook = lambda h: setattr(mod, "_hook", h)
            mod.get_axon_ntff_profile_hook = lambda: mod._hook
            sys.modules["antenv.axon_hooks"] = mod
            antenv.axon_hooks = mod
            from trn_agent_boot.trn_boot import _ntff_profile_via_ctypes

            mod.set_axon_ntff_profile_hook(
                _ntff_profile_via_ctypes("/opt/axon/libaxon_pjrt.so")
            )

        _orig_upload = _bu.upload_artifacts

        def _safe_upload(tmpdir):
            try:
                return _orig_upload(tmpdir)
            except Exception:
                return "local://" + str(tmpdir)

        _bu.upload_artifacts = _safe_upload
    except Exception:
        pass


_install_trace_shims()

B, S, KD, QD = 4096, 200, 128, 128
NCORES = 8
P = 128
NSLOTS = B // NCORES          # 512 slots (batches) per core
KAUG = 130                    # keys + mask elem + pad elem (4B-aligned rows)
BIGNEG = -30000.0
SPLIT = 224                   # slots [0, SPLIT) on PE; rest on DVE tiles
QWC = KD + NSLOTS + P         # qw cols (zero pad so partial tiles slice OK)

# consumption-rate model for the DMA feed schedule (ns)
T0_PE, RATE_PE_COL = 9000.0, 1.35
T0_DVE, RATE_DVE_POS = 11000.0, 225.0

LAST_RESULTS = None
_nc_cache = {}


def _chunk_sizes(total, ramp, steady):
    sizes, c = [], 0
    for r in ramp:
        if c >= total:
            break
        s = min(r, total - c)
        sizes.append(s)
        c += s
    while c < total:
        s = min(steady, total - c)
        sizes.append(s)
        c += s
    return sizes


def _dve_tiles(exts):
    tiles = []
    s0 = SPLIT
    while s0 < NSLOTS:
        s1 = min(s0 + P, NSLOTS)
        tiles.append((s0, s1, int(exts[s0])))
        s0 = s1
    return tiles


def _build(exts):
    exts = list(exts)
    f32, f16 = mybir.dt.float32, mybir.dt.float16
    Exp = mybir.ActivationFunctionType.Exp

    pe_slots = list(range(SPLIT))
    los = {j: min(exts[j], P) for j in pe_slots}
    his = {j: exts[j] - los[j] for j in pe_slots}
    offs = {}
    c = 0
    for j in pe_slots:
        offs[j] = c
        c += exts[j]
    TOTP = c
    pe_groups = [(0, P), (P, SPLIT)]
    gh = [max(his[j] for j in range(a, b)) for (a, b) in pe_groups]

    tiles = _dve_tiles(exts)
    NT = len(tiles)
    TOTV = sum(E for (_, _, E) in tiles)

    nc = bacc.Bacc("TRN2", target_bir_lowering=False, debug=False)
    kv_d = nc.dram_tensor("kv", [P, TOTV, KAUG], f16, kind="ExternalInput")
    kt_d = nc.dram_tensor("kt", [P, TOTP], f16, kind="ExternalInput")
    qw_d = nc.dram_tensor("qw", [P, QWC], f16, kind="ExternalInput")
    cn_d = nc.dram_tensor("cn", [1, 2 * P], f32, kind="ExternalInput")
    op_d = nc.dram_tensor("op", [2, S, P], f32, kind="ExternalOutput")
    ov_d = nc.dram_tensor("ov", [NT, P, S], f32, kind="ExternalOutput")

    # --- chunk plans with deadline-ordered feed -------------------------
    kt_sizes = _chunk_sizes(TOTP, [512, 1024, 2048, 3072], 4096)
    kt_chunks = []  # (j0, j1, c0, c1)
    ji, c = 0, 0
    for tgt in kt_sizes:
        j0, c0 = ji, c
        while ji < SPLIT and c - c0 < tgt:
            c += exts[ji]
            ji += 1
        kt_chunks.append((j0, ji, c0, c))
        if ji >= SPLIT:
            break
    slot_chunk = {}
    for ci, (j0, j1, c0, c1) in enumerate(kt_chunks):
        for j in range(j0, j1):
            slot_chunk[j] = (ci, c0)

    kv_sizes = _chunk_sizes(TOTV, [16, 24, 32, 40], 48)
    kv_chunks = []
    p0 = 0
    for s_ in kv_sizes:
        kv_chunks.append((p0, s_))
        p0 += s_

    feed = []
    for ci, (j0, j1, c0, c1) in enumerate(kt_chunks):
        feed.append((T0_PE + RATE_PE_COL * c0, "kt", ci))
    for ci, (p0, npos) in enumerate(kv_chunks):
        feed.append((T0_DVE + RATE_DVE_POS * p0, "kv", ci))
    feed.sort(key=lambda x: x[0])

    with tile.TileContext(nc) as tc:
        with (
            tc.tile_pool(name="ktp", bufs=1) as ktp,
            tc.tile_pool(name="kvp", bufs=1) as kvp,
            tc.tile_pool(name="small", bufs=1) as smallp,
            tc.tile_pool(name="scr", bufs=16) as scrp,
            tc.tile_pool(name="psum", bufs=1, space=bass.MemorySpace.PSUM) as psump,
        ):
            qw = smallp.tile([P, QWC], f16, tag="qw")
            nc.sync.dma_start(qw[:], qw_d[:])
            cn = smallp.tile([1, 2 * P], f32, tag="cn")
            nc.gpsimd.dma_start(cn[:], cn_d[:])

            kt_tiles = [None] * len(kt_chunks)
            kv_tiles = [None] * len(kv_chunks)
            for _, kind, ci in feed:
                if kind == "kt":
                    j0, j1, c0, c1 = kt_chunks[ci]
                    t = ktp.tile([P, c1 - c0], f16, tag=f"kt{ci}",
                                 name=f"kt{ci}")
                    nc.sync.dma_start(t[:], kt_d[:, c0:c1])
                    kt_tiles[ci] = t
                else:
                    p0, npos = kv_chunks[ci]
                    t = kvp.tile([P, npos, KAUG], f16, tag=f"kv{ci}",
                                 name=f"kv{ci}")
                    nc.sync.dma_start(t[:], kv_d[:, p0 : p0 + npos, :])
                    kv_tiles[ci] = t

            # --- qp projections: DVE tiles first (gate the DVE stream)
            # PSUM is bank-granular (8 x 2KB): qpb uses a 2-buf ring and
            # qpt shares a tile with the late-used recb outer product.
            qpb_ps = [psump.tile([P, P], f32, tag="qpb", bufs=2,
                                 name=f"qpb{t}")
                      for t in range(NT)]
            for t, (s0, s1, E) in enumerate(tiles):
                nc.tensor.matmul(
                    qpb_ps[t][:], qw[:, KD + s0 : KD + s0 + P], qw[:, 0:KD],
                    start=True, stop=True,
                )
            mix_ps = psump.tile([P, 2 * P], f32, tag="mix")
            qpt_ps = mix_ps[:, 0:SPLIT]
            nc.tensor.matmul(
                qpt_ps, qw[:, 0:KD], qw[:, KD : KD + SPLIT],
                start=True, stop=True,
            )

            qp_aug = [smallp.tile([P, KAUG], f16, tag=f"qpa{t}", name=f"qpa{t}")
                      for t in range(NT)]
            for t in range(NT):
                nc.scalar.copy(qp_aug[t][:, 0:KD], qpb_ps[t][:])
                nc.vector.memset(qp_aug[t][:, KD : KD + 1], 1.0)
                nc.vector.memset(qp_aug[t][:, KD + 1 : KAUG], 0.0)
            qpt = smallp.tile([P, SPLIT], f16, tag="qpt")
            nc.scalar.copy(qpt[:], qpt_ps)

            ones_col = smallp.tile([P, 1], f32, tag="ones_col")
            nc.vector.memset(ones_col[:], 1.0)
            ones_row = smallp.tile([1, P], f32, tag="ones_row")
            nc.vector.memset(ones_row[:], 1.0)

            sc_lo = [psump.tile([P, P], f32, tag=f"sclo{i}", name=f"sclo{i}")
                     for i in range(2)]
            sc_hi = [psump.tile([P, P], f32, tag=f"schi{i}", name=f"schi{i}")
                     for i in range(2)]
            for t in sc_lo + sc_hi:
                nc.vector.memset(t[:], 0.0)

            ssum_ps = psump.tile([1, 2 * P], f32, tag="ssum")
            recb_ps = mix_ps
            ssc = smallp.tile([1, 2 * P], f32, tag="ssc")
            rec_t = smallp.tile([1, 2 * P], f32, tag="rec")
            em_lo = [smallp.tile([P, P], f32, tag=f"emlo{i}", name=f"emlo{i}")
                     for i in range(2)]
            em_hi = [smallp.tile([P, P], f32, tag=f"emhi{i}", name=f"emhi{i}")
                     for i in range(2)]
            o_lo = [smallp.tile([P, P], f32, tag=f"olo{i}", name=f"olo{i}")
                    for i in range(2)]
            o_hi = [smallp.tile([P, P], f32, tag=f"ohi{i}", name=f"ohi{i}")
                    for i in range(2)]

            att = [smallp.tile([P, E], f32, tag=f"att{t}", name=f"att{t}")
                   for t, (_, _, E) in enumerate(tiles)]
            tile_base = {}
            base = 0
            for t, (_, _, E) in enumerate(tiles):
                tile_base[t] = base
                base += E

            def emit_dve_stts(t):
                base = tile_base[t]
                E = tiles[t][2]
                for ci, (p0, npos) in enumerate(kv_chunks):
                    lo = max(p0, base)
                    hi = min(p0 + npos, base + E)
                    ck = kv_tiles[ci]
                    for p in range(lo, hi):
                        scr = scrp.tile([P, KAUG], f32, tag="scr")
                        nc.vector.scalar_tensor_tensor(
                            scr[:],
                            ck[:, p - p0, :],
                            1.0,
                            qp_aug[t][:],
                            op0=mybir.AluOpType.mult,
                            op1=mybir.AluOpType.mult,
                            accum_out=att[t][:, p - base : p - base + 1],
                        )

            def emit_dve_softmax(t):
                E = tiles[t][2]
                e_t = smallp.tile([P, E], f32, tag=f"e{t}", name=f"e{t}")
                ssumv = smallp.tile([P, 1], f32, tag=f"ssv{t}", name=f"ssv{t}")
                nc.scalar.activation(
                    e_t[:], att[t][:], Exp, bias=0.0, scale=1.0,
                    accum_out=ssumv[:],
                )
                recv = smallp.tile([P, 1], f32, tag=f"rcv{t}", name=f"rcv{t}")
                nc.vector.reciprocal(recv[:], ssumv[:])
                o_t = smallp.tile([P, E], f32, tag=f"ovt{t}", name=f"ovt{t}")
                nc.scalar.mul(o_t[:], e_t[:], recv[:])
                nc.sync.dma_start(ov_d[t, :, 0:E], o_t[:])

            def emit_pe_mms(g):
                a, b_ = pe_groups[g]
                for j in range(a, b_):
                    col = j - a
                    ci, c0 = slot_chunk[j]
                    o = offs[j] - c0
                    ck = kt_tiles[ci]
                    nc.tensor.matmul(
                        sc_lo[g][0 : los[j], col : col + 1],
                        ck[:, o : o + los[j]],
                        qpt[:, j : j + 1],
                        start=True, stop=True,
                    )
                    if his[j] > 0:
                        nc.tensor.matmul(
                            sc_hi[g][0 : his[j], col : col + 1],
                            ck[:, o + P : o + P + his[j]],
                            qpt[:, j : j + 1],
                            start=True, stop=True,
                        )

            def emit_pe_softmax_a(g):
                el = em_lo[g]
                gcols = slice(g * P, (g + 1) * P)
                nc.scalar.activation(el[:], sc_lo[g][:], Exp,
                                     bias=0.0, scale=1.0)
                if gh[g] > 0:
                    nc.scalar.activation(em_hi[g][0 : gh[g], :],
                                         sc_hi[g][0 : gh[g], :],
                                         Exp, bias=0.0, scale=1.0)
                nc.tensor.matmul(
                    ssum_ps[0:1, gcols], ones_col[0:P, 0:1], el[:],
                    start=True, stop=(gh[g] == 0),
                )
                if gh[g] > 0:
                    nc.tensor.matmul(
                        ssum_ps[0:1, gcols], ones_col[0 : gh[g], 0:1],
                        em_hi[g][0 : gh[g], :],
                        start=False, stop=True,
                    )

            def emit_pe_softmax_b(g):
                ol = o_lo[g]
                gcols = slice(g * P, (g + 1) * P)
                nc.vector.tensor_sub(ssc[0:1, gcols], ssum_ps[0:1, gcols],
                                     cn[0:1, gcols])
                nc.vector.reciprocal_approx_fast(rec_t[0:1, gcols],
                                                 ssc[0:1, gcols])
                nc.tensor.matmul(
                    recb_ps[:, gcols], ones_row[0:1, 0:P], rec_t[0:1, gcols],
                    start=True, stop=True,
                )
                nc.vector.tensor_mul(ol[:], em_lo[g][:], recb_ps[:, gcols])
                nc.sync.dma_start(op_d[g, 0:P, :], ol[:])
                if gh[g] > 0:
                    nc.vector.tensor_mul(o_hi[g][0 : gh[g], :],
                                         em_hi[g][0 : gh[g], :],
                                         recb_ps[0 : gh[g], gcols])
                    nc.sync.dma_start(op_d[g, P : P + gh[g], :],
                                      o_hi[g][0 : gh[g], :])

            # --- emission schedule --------------------------------------
            emit_pe_mms(0)
            emit_dve_stts(0)
            emit_pe_mms(1)
            emit_pe_softmax_a(0)
            emit_pe_softmax_b(0)
            emit_dve_softmax(0)
            emit_dve_stts(1)
            emit_pe_softmax_a(1)
            emit_pe_softmax_b(1)
            emit_dve_softmax(1)
            if NT > 2:
                emit_dve_stts(2)
                emit_dve_softmax(2)

    nc.compile()
    return nc


def _prep(query, keys, seq_len, w):
    query = np.asarray(query)
    keys = np.asarray(keys)
    w = np.asarray(w)
    lens = np.asarray(seq_len).reshape(B).astype(np.int64)

    order = np.argsort(-lens, kind="stable")
    exts = np.maximum(1, np.minimum(S, lens[order[0::NCORES]])).astype(int)

    pe_slots = list(range(SPLIT))
    offs = {}
    c = 0
    for j in pe_slots:
        offs[j] = c
        c += int(exts[j])
    TOTP = c
    pe_groups = [(0, P), (P, SPLIT)]
    gh = [max(max(int(exts[j]) - P, 0) for j in range(a, b))
          for (a, b) in pe_groups]

    tiles = _dve_tiles(exts)
    TOTV = sum(E for (_, _, E) in tiles)

    keys16 = np.ascontiguousarray(keys, dtype=np.float16)
    q32 = query[:, 0, :].astype(np.float32)
    qp32 = q32 @ w.astype(np.float32).T
    q16 = np.ascontiguousarray(q32, dtype=np.float16)
    wT16 = np.ascontiguousarray(w.T, dtype=np.float16)

    in_maps = []
    perms = []
    for cidx in range(NCORES):
        idx = order[cidx::NCORES]
        perms.append(idx)
        l_c = lens[idx]

        kv = np.zeros((P, TOTV, KAUG), dtype=np.float16)
        base = 0
        for (s0, s1, E) in tiles:
            n = s1 - s0
            bidx = idx[s0:s1]
            kv[:n, base : base + E, :KD] = keys16[bidx, :E, :]
            svE = np.arange(E)[None, :]
            kv[:n, base : base + E, KD] = np.where(
                svE < l_c[s0:s1][:, None], 0.0, np.float16(BIGNEG)
            )
            if n < P:
                kv[n:, base : base + E, KD] = np.float16(BIGNEG)
            base += E

        kt = np.zeros((P, TOTP), dtype=np.float16)
        for j in pe_slots:
            e = int(exts[j])
            b = idx[j]
            o = offs[j]
            kt[:, o : o + e] = keys16[b, :e, :].T
            ln = int(l_c[j])
            if ln < e:
                qpb = qp32[b]
                mcol = (qpb * (BIGNEG / float(qpb @ qpb))).astype(np.float16)
                kt[:, o + ln : o + e] = mcol[:, None]

        qw = np.zeros((P, QWC), dtype=np.float16)
        qw[:, :KD] = wT16
        qw[:, KD : KD + NSLOTS] = q16[idx].T

        cnv = np.zeros((1, 2 * P), dtype=np.float32)
        for g, (a, b_) in enumerate(pe_groups):
            for j in range(a, b_):
                e = int(exts[j])
                lo_stale = P - min(e, P)
                hi_stale = gh[g] - max(e - P, 0) if gh[g] > 0 else 0
                cnv[0, g * P + (j - a)] = lo_stale + hi_stale

        in_maps.append({"kv": kv, "kt": kt, "qw": qw, "cn": cnv})
    return lens, exts, perms, in_maps


def kernel(query, keys, seq_len, w):
    global LAST_RESULTS
    lens, exts, perms, in_maps = _prep(query, keys, seq_len, w)

    key = tuple(int(e) for e in exts)
    nc = _nc_cache.get(key)
    if nc is None:
        nc = _build(key)
        _nc_cache[key] = nc

    res = run_bass_kernel_spmd(nc, in_maps, core_ids=list(range(NCORES)))
    LAST_RESULTS = res

    tiles = _dve_tiles(exts)
    out = np.zeros((B, S), dtype=np.float32)
    sv = np.arange(S)[None, :]
    for c in range(NCORES):
        dev_p = np.asarray(res.results[c]["op"])   # [2, S, P]
        dev_v = np.asarray(res.results[c]["ov"])   # [NT, P, S]
        idx = perms[c]
        l_c = lens[idx]

        full = np.zeros((NSLOTS, S), dtype=np.float32)
        full[0:P, :] = dev_p[0].T
        full[P:SPLIT, :] = dev_p[1].T[: SPLIT - P]
        for t, (s0, s1, E) in enumerate(tiles):
            full[s0:s1, :] = dev_v[t][: s1 - s0]

        arr = np.where(sv < l_c[:, None], full, 0.0).astype(np.float32)
        arr[l_c == 0] = np.float32(1.0 / S)
        out[idx] = arr
    return out
